# revision 47
# baseline (speedup 1.0000x reference)
"""Distributed Trainium2 Bass kernel for nn_AttentionLayer_25993142075512.

Sharding: 8 cores = 2 batches x 4 head-groups (4 heads each). Each core
computes its batch's q/k/v projections for its 4 heads, causal attention,
and a partial output projection o @ Wo[head_rows]. Host sums the 4
partials per batch and adds bo (plus the folded v-bias term).

v2.1 design notes:
  - qT[pr]/kT[pr] hold a HEAD PAIR: head 2pr at partitions 0:64, head 2pr+1
    at 64:128 (natural dim order). Score matmuls for the two heads go to PE
    row tiles (0,0)/(64,0) and run CONCURRENTLY (2x row tiling).
  - RoPE via pre-rotated projection copies: q2 = rot2(Wq)^T s (host-prepped
    swapped/negated weight columns), so rope = q*cos + q2*sin: 3 DVE ops per
    head per chunk, all partition bases 0 mod 32.
  - vT produced directly in [keys, dims] layout (stationary = s_kv^T chunk,
    moving = Wv columns) - no transpose phase. v bias folded into bo on host.
  - oT matmuls K-split into two 64-key halves on row tiles T0/T8 so the
    whole attention inner loop stays in (64,128) PE tiling mode (no
    mode-switch drains); softmax denominator via a ones column (M=65).
  - exp for both heads fused in one ACT op [128, 2, w] over two psum banks.
  - Denominators summed into a free-dim-indexed [1,16,512] tile (engine
    partition bases must be 0 mod 32), spread by SBUF->SBUF DMA, one
    reciprocal, broadcast via one-hot E matmuls (64-mode), DVE multiply.
  - Attention groups of pair 0 interleaved into the projection sub loop;
    PSUM: proj 2x2KB + sT 4KB + oT 4x2KB = 16KB exactly.
  - Consolidated host-packed weight DMAs; skvt input DMAs issued from the
    (otherwise idle) gpsimd queue to halve Sync descriptor-issue time.

Assumes mask_q == 1 (spec fill=ones); mask_kv handled exactly via exp bias.
"""

import sys, os, types, ctypes, contextlib

sys.path.insert(0, "/opt/trn_rl_repo")

import numpy as np
import ml_dtypes


def _install_axon_hooks():
    so = "/opt/axon/libaxon_pjrt.so"

    def _hook_factory(so_path):
        if not os.path.exists(so_path):
            return None
        lib = ctypes.CDLL(so_path)
        if not hasattr(lib, "axon_start_nrt_profile"):
            return None
        lib.axon_start_nrt_profile.argtypes = [
            ctypes.POINTER(ctypes.c_int64),
            ctypes.c_size_t,
        ]
        lib.axon_start_nrt_profile.restype = ctypes.c_int64
        lib.axon_stop_nrt_profile.argtypes = [ctypes.c_char_p]
        lib.axon_stop_nrt_profile.restype = ctypes.c_int64

        @contextlib.contextmanager
        def _hook(output_dir, device_ids):
            import jax

            jax.devices()
            if device_ids:
                ids = (ctypes.c_int64 * len(device_ids))(*device_ids)
                rc = lib.axon_start_nrt_profile(ids, len(device_ids))
            else:
                rc = lib.axon_start_nrt_profile(None, 0)
            if rc != 0:
                raise RuntimeError(f"axon_start_nrt_profile rc={rc}")
            try:
                yield
            finally:
                n = lib.axon_stop_nrt_profile(str(output_dir).encode())
                if n < 0:
                    raise RuntimeError(f"axon_stop_nrt_profile rc={n}")

        return _hook

    try:
        import antenv

        if "antenv.axon_hooks" not in sys.modules:
            hook = _hook_factory(so)
            mod = types.ModuleType("antenv.axon_hooks")
            mod.get_axon_ntff_profile_hook = lambda: hook
            mod.set_axon_ntff_profile_hook = lambda h: None
            antenv.axon_hooks = mod
            sys.modules["antenv.axon_hooks"] = mod
    except ImportError:
        pass
    from concourse import bass_utils

    bass_utils.upload_artifacts = lambda tmpdir: tmpdir


_install_axon_hooks()

from concourse import bass, bacc, tile, mybir  # noqa: E402

BF16 = mybir.dt.bfloat16
F32 = mybir.dt.float32
NPBF16 = ml_dtypes.bfloat16

B, N, DQ, DKV, H, DH, DOUT = 2, 2048, 1024, 1024, 16, 64, 1024
ROT = DH // 2  # 32
INF = 1.0e6
HPC = 4  # heads per core (2 pairs)
NB = N // 128  # 16 k-blocks
NG = NB // 4  # 4 q-block groups (512 cols each)
NS = 4  # projection subs (512 cols each)
VS = 66  # vg per-(kb,pr,hh) stride (64 v cols + ones col + pad)


def build_nc():
    nc = bacc.Bacc(None, target_bir_lowering=False)

    sqt_d = nc.declare_dram_parameter("sqt", [DQ, N], BF16, isOutput=False)
    skvt_d = nc.declare_dram_parameter("skvt", [DKV, N], BF16, isOutput=False)
    wq_d = nc.declare_dram_parameter("wq", [2, 128, 1024], BF16, isOutput=False)
    wk_d = nc.declare_dram_parameter("wk", [2, 128, 1024], BF16, isOutput=False)
    wq2_d = nc.declare_dram_parameter("wq2", [128, 1024], BF16, isOutput=False)
    wk2_d = nc.declare_dram_parameter("wk2", [128, 1024], BF16, isOutput=False)
    wv_d = nc.declare_dram_parameter("wv", [128, 2048], BF16, isOutput=False)
    wo_d = nc.declare_dram_parameter("wo", [2, 128, DOUT], BF16, isOutput=False)
    bq_d = nc.declare_dram_parameter("bq", [128, 2], F32, isOutput=False)
    bk_d = nc.declare_dram_parameter("bk", [128, 2], F32, isOutput=False)
    bq2_d = nc.declare_dram_parameter("bq2", [128, 1], F32, isOutput=False)
    bk2_d = nc.declare_dram_parameter("bk2", [128, 1], F32, isOutput=False)
    cost_d = nc.declare_dram_parameter("cost", [128, N], BF16, isOutput=False)
    sint_d = nc.declare_dram_parameter("sint", [128, N], BF16, isOutput=False)
    mtile_d = nc.declare_dram_parameter("mtile", [128, 2, 128], BF16, isOutput=False)
    e64_d = nc.declare_dram_parameter("e64", [64, 1024], BF16, isOutput=False)
    bmask_d = nc.declare_dram_parameter("bmask", [128, NB], F32, isOutput=False)
    out_ext = nc.declare_dram_parameter("out", [N, DOUT], BF16, isOutput=True)

    AF = mybir.ActivationFunctionType
    ALU = mybir.AluOpType

    with tile.TileContext(nc) as tc:
        with (
            tc.tile_pool(name="const", bufs=1) as cpool,
            tc.tile_pool(name="big", bufs=1) as bigpool,
            tc.tile_pool(name="small", bufs=8) as smallpool,
            tc.tile_pool(name="ptile", bufs=4) as ppool,
            tc.tile_pool(name="outsb", bufs=4) as outsb_pool,
        ):
            # ---- SBUF constants ----
            wq_sb = [cpool.tile([128, 1024], BF16, tag=f"wq{p}", name=f"wq{p}") for p in range(2)]
            wk_sb = [cpool.tile([128, 1024], BF16, tag=f"wk{p}", name=f"wk{p}") for p in range(2)]
            wq2_sb = cpool.tile([128, 1024], BF16, tag="wq2", name="wq2")
            wk2_sb = cpool.tile([128, 1024], BF16, tag="wk2", name="wk2")
            wv_sb = cpool.tile([128, 8, 256], BF16, tag="wv", name="wv")
            wo_sb = [cpool.tile([128, DOUT], BF16, tag=f"wo{p}", name=f"wo{p}") for p in range(2)]
            bq_sb = cpool.tile([128, 2], F32, tag="bq", name="bq")
            bk_sb = cpool.tile([128, 2], F32, tag="bk", name="bk")
            bq2_sb = cpool.tile([128, 1], F32, tag="bq2", name="bq2")
            bk2_sb = cpool.tile([128, 1], F32, tag="bk2", name="bk2")
            cost = cpool.tile([128, N], BF16, tag="cost", name="cost")
            sint = cpool.tile([128, N], BF16, tag="sint", name="sint")
            mtile = cpool.tile([128, 2, 128], BF16, tag="mtile", name="mtile")
            e64 = cpool.tile([64, 8, 128], BF16, tag="e64", name="e64")
            bmask = cpool.tile([128, NB], F32, tag="bmask", name="bmask")

            # full-resident transposed inputs, 8 chunks of 128 dq-dims each
            sqt = [bigpool.tile([128, N], BF16, tag=f"sqt{c}", name=f"sqt{c}") for c in range(8)]
            skvt = [bigpool.tile([128, N], BF16, tag=f"skvt{c}", name=f"skvt{c}") for c in range(8)]

            # consolidated constant DMAs; sqt on sync, skvt on gpsimd so the
            # two descriptor queues and transfer streams run in parallel.
            # Minimal proj prerequisites first, big tables after the inputs.
            nc.sync.dma_start(bq_sb[:], bq_d[:])
            nc.sync.dma_start(bk_sb[:], bk_d[:])
            nc.sync.dma_start(bq2_sb[:], bq2_d[:])
            nc.sync.dma_start(bk2_sb[:], bk2_d[:])
            for p in range(2):
                nc.sync.dma_start(wq_sb[p][:], wq_d[p])
                nc.gpsimd.dma_start(wk_sb[p][:], wk_d[p])
            nc.sync.dma_start(wq2_sb[:], wq2_d[:])
            nc.gpsimd.dma_start(wk2_sb[:], wk2_d[:])
            nc.gpsimd.dma_start(wv_sb[:], wv_d[:])
            for hf in range(2):
                hs = slice(hf * 1024, (hf + 1) * 1024)
                for c in range(8):
                    eng = nc.sync if c % 2 == 0 else nc.scalar
                    eng.dma_start(sqt[c][:, hs], sqt_d[c * 128 : (c + 1) * 128, hs])
                    nc.gpsimd.dma_start(skvt[c][:, hs], skvt_d[c * 128 : (c + 1) * 128, hs])
                if hf == 0:
                    nc.sync.dma_start(cost[:], cost_d[:])
                    nc.sync.dma_start(sint[:], sint_d[:])

            def _late_const_dmas():
                nc.sync.dma_start(mtile[:], mtile_d[:])
                nc.sync.dma_start(e64[:], e64_d[:])
                nc.sync.dma_start(bmask[:], bmask_d[:])
                for p in range(2):
                    nc.sync.dma_start(wo_sb[p][:], wo_d[p])

            # ---- persistent activations ----
            qT = [bigpool.tile([128, N], BF16, tag=f"qT{p}", name=f"qT{p}") for p in range(2)]
            kT = [bigpool.tile([128, N], BF16, tag=f"kT{p}", name=f"kT{p}") for p in range(2)]
            # rotate_every_two copies: head (pr,hh) rot rows at 64*pr+32*hh
            q2all = bigpool.tile([128, N], BF16, tag="q2all", name="q2all")
            k2all = bigpool.tile([128, N], BF16, tag="k2all", name="k2all")
            # vgAll[:, kb, pr, hh, 0:64] = v of head 2pr+hh for key block kb,
            # [:, kb, pr, hh, 64] = ones (denominator column)
            vgAll = bigpool.tile([128, NB, 2, 2, VS], BF16, tag="vg", name="vg")
            oTs = [
                [bigpool.tile([128, 512], BF16, tag=f"oTs{p}_{g}", name=f"oTs{p}_{g}") for g in range(NG)]
                for p in range(2)
            ]
            dsum = bigpool.tile([1, 16, 512], F32, tag="dsum", name="dsum")
            denoms = bigpool.tile([16, 512], F32, tag="denoms", name="denoms")
            rec = bigpool.tile([16, 512], F32, tag="rec", name="rec")
            recb = bigpool.tile([64, 512], BF16, tag="recb", name="recb")

            # hoisted memsets (head of DVE queue); denoms=1.0 so the padded
            # reciprocal rows (alignment rule) stay finite (0*NaN poisons psum)
            nc.vector.memset(vgAll[:], 1.0)
            nc.vector.memset(recb[:], 0.0)
            nc.vector.memset(denoms[:], 1.0)

            def rope_block(dst, dst2, pr, hh, c0, cw):
                """out = q*cos + q2*sin on dst[64*hh:64*hh+32, c0:c0+cw]."""
                cs = slice(c0, c0 + cw)
                r = 64 * hh
                r2 = 64 * pr + 32 * hh
                t1 = smallpool.tile([32, cw], BF16, tag="ropet1", name="ropet1", bufs=2)
                t2 = smallpool.tile([32, cw], BF16, tag="ropet2", name="ropet2", bufs=2)
                v = nc.vector
                v.tensor_mul(t2[:, :], dst2[r2 : r2 + 32, cs], sint[r2 : r2 + 32, cs])
                v.tensor_mul(t1[:, :], dst[r : r + 32, cs], cost[r : r + 32, cs])
                v.tensor_add(dst[r : r + 32, cs], t1[:, :], t2[:, :])

            def attn_group(pr, g, stq, otq):
                """Attention for head pair pr, query group g. All matmuls in
                (64,128) tiling mode; head pair + K-halves run concurrently."""
                oT = [
                    otq.tile([128, 512], F32, tag=f"oT{hh}", name=f"oT{hh}")
                    for hh in range(2)
                ]
                for kb in range(4 * g + 4):
                    q0 = max(kb, 4 * g)
                    off = (q0 % 4) * 128
                    qs = slice(g * 512 + off, (g + 1) * 512)
                    ks = slice(kb * 128, (kb + 1) * 128)
                    sT = stq.tile([128, 2, 512], F32, tag="sT", name="sT", bufs=2)
                    nc.tensor.matmul(
                        sT[:, 0, off:], kT[pr][0:64, ks], qT[pr][0:64, qs],
                        start=True, stop=True,
                    )
                    nc.tensor.matmul(
                        sT[:, 1, off:], kT[pr][64:128, ks], qT[pr][64:128, qs],
                        start=True, stop=True,
                    )
                    p = ppool.tile([128, 2, 512], BF16, tag="p", name="p")
                    nc.scalar.activation(
                        p[:, :, off:], sT[:, :, off:], AF.Exp,
                        bias=bmask[:, kb : kb + 1], scale=0.125,
                    )
                    if q0 == kb:  # diagonal block: zero the upper triangle of
                        # exp on the (idle) gpsimd engine - keeps the DVE queue
                        # out of the exp->oT critical chain
                        nc.gpsimd.tensor_mul(
                            p[:, :, off : off + 128], p[:, :, off : off + 128], mtile[:]
                        )
                    st = kb == 0
                    sp = kb == 4 * g + 3
                    for hh in range(2):
                        nc.tensor.matmul(
                            oT[hh][0:65, off:], vgAll[:, kb, pr, hh, 0:65],
                            p[:, hh, off:], start=st, stop=sp,
                        )
                # evacuate unnormalized o and denominator sums
                for hh in range(2):
                    idx = pr * 8 + g * 2 + hh
                    nc.vector.tensor_copy(
                        oTs[pr][g][hh * 64 : hh * 64 + 64, :], oT[hh][0:64, :]
                    )
                    nc.vector.tensor_copy(dsum[0:1, idx, :], oT[hh][64:65, :])

            # ============ phases 1-3: projections + attention, interleaved ============
            with (
                tc.tile_pool(name="pjps", bufs=2, space=bass.MemorySpace.PSUM) as pj,
                tc.tile_pool(name="stps", bufs=1, space=bass.MemorySpace.PSUM) as stq,
                tc.tile_pool(name="otps", bufs=1, space=bass.MemorySpace.PSUM) as otq,
            ):
                def proj_sub(s):
                    cs = slice(s * 512, (s + 1) * 512)
                    projs = [
                        (wq_sb[0], bq_sb[:, 0:1], qT[0], sqt),
                        (wq_sb[1], bq_sb[:, 1:2], qT[1], sqt),
                        (wq2_sb, bq2_sb[:], q2all, sqt),
                        (wk_sb[0], bk_sb[:, 0:1], kT[0], skvt),
                        (wk_sb[1], bk_sb[:, 1:2], kT[1], skvt),
                        (wk2_sb, bk2_sb[:], k2all, skvt),
                    ]
                    for pi, (wsb, bsb, dst, src) in enumerate(projs):
                        ps = pj.tile([128, 512], F32, tag="pj", name="pj")
                        for c in range(8):
                            nc.tensor.matmul(
                                ps[:],
                                wsb[:, c * 128 : (c + 1) * 128],
                                src[c][:, cs],
                                start=(c == 0), stop=(c == 7),
                            )
                        # split evacuations between ACT (mostly idle in proj)
                        # and DVE to keep both off the critical path
                        if pi % 3 == 2:
                            nc.vector.tensor_scalar(dst[:, cs], ps[:], bsb, None, ALU.add)
                        else:
                            nc.scalar.activation(dst[:, cs], ps[:], AF.Identity, bias=bsb)
                    # vT for this sub's 4 key blocks, two blocks per psum bank
                    for half in range(2):
                        pv = pj.tile([128, 512], F32, tag="pj", name="pj")
                        for kl in range(2):
                            kb = 4 * s + 2 * half + kl
                            ks = slice(kb * 128, (kb + 1) * 128)
                            for c in range(8):
                                nc.tensor.matmul(
                                    pv[:, kl * 256 : (kl + 1) * 256],
                                    skvt[c][:, ks],
                                    wv_sb[:, c, :],
                                    start=(c == 0 and kl == 0),
                                    stop=(c == 7 and kl == 1),
                                )
                        for kl in range(2):
                            kb = 4 * s + 2 * half + kl
                            if kl == 0:
                                nc.scalar.activation(
                                    vgAll[:, kb, :, :, 0:64],
                                    pv[:, kl * 256 : (kl + 1) * 256],
                                    AF.Copy,
                                )
                            else:
                                nc.vector.tensor_copy(
                                    vgAll[:, kb, :, :, 0:64],
                                    pv[:, kl * 256 : (kl + 1) * 256],
                                )

                proj_sub(0)
                _late_const_dmas()
                proj_sub(1)
                for p in range(2):
                    for hh in range(2):
                        rope_block(qT[p], q2all, p, hh, 0, 1024)
                        rope_block(kT[p], k2all, p, hh, 0, 1024)
                attn_group(0, 0, stq, otq)
                proj_sub(2)
                for p in range(2):
                    for hh in range(2):
                        rope_block(qT[p], q2all, p, hh, 1024, 512)
                        rope_block(kT[p], k2all, p, hh, 1024, 512)
                attn_group(0, 1, stq, otq)
                proj_sub(3)
                for p in range(2):
                    for hh in range(2):
                        rope_block(qT[p], q2all, p, hh, 1536, 512)
                        rope_block(kT[p], k2all, p, hh, 1536, 512)
                attn_group(0, 2, stq, otq)
                attn_group(0, 3, stq, otq)
                def norm_pair0(g):
                    bc = pj.tile([128, 512], F32, tag="pj", name="pj")
                    nc.tensor.matmul(
                        bc[:], e64[:, g, :], recb[:], start=True, stop=True,
                        tile_position=(0, 0),
                    )
                    nc.vector.tensor_mul(oTs[0][g][:], oTs[0][g][:], bc[:])

                def norm1_and_outproj(g):
                    bc = pj.tile([128, 512], F32, tag="pj", name="pj")
                    nc.tensor.matmul(
                        bc[:], e64[:, 4 + g, :], recb[:], start=True, stop=True,
                        tile_position=(0, 0),
                    )
                    nc.vector.tensor_mul(oTs[1][g][:], oTs[1][g][:], bc[:])
                    for qb in range(4 * g, 4 * g + 4):
                        off = (qb % 4) * 128
                        ob = outsb_pool.tile([128, DOUT], BF16, tag="ob", name="ob")
                        for nh in range(2):
                            po = pj.tile([128, 512], F32, tag="pj", name="pj")
                            for p in range(2):
                                nc.tensor.matmul(
                                    po[:],
                                    oTs[p][g][:, off : off + 128],
                                    wo_sb[p][:, nh * 512 : (nh + 1) * 512],
                                    start=(p == 0), stop=(p == 1),
                                )
                            half = ob[:, nh * 512 : (nh + 1) * 512]
                            if (qb + nh) % 2 == 0:
                                nc.scalar.activation(half, po[:], AF.Copy)
                            else:
                                nc.vector.tensor_copy(half, po[:])
                        nc.gpsimd.dma_start(out_ext[qb * 128 : (qb + 1) * 128, :], ob[:])

                # pair-0 denominators: gather + reciprocal + normalize early,
                # overlapping pair-1 attention (bc reuses freed proj psum)
                nc.sync.dma_start(denoms[0:8, :], dsum[0:1, 0:8, :])
                nc.vector.reciprocal(rec[0:8, :], denoms[0:8, :])
                nc.vector.tensor_copy(recb[0:8, :], rec[0:8, :])
                attn_group(1, 0, stq, otq)
                norm_pair0(0)
                attn_group(1, 1, stq, otq)
                norm_pair0(1)
                # groups 0/1 of pair 1 are done: reciprocal over rows 0:12
                # padded to 0:16 (alignment rule; rows 12:16 garbage, unused)
                nc.sync.dma_start(denoms[8:12, :], dsum[0:1, 8:12, :])
                nc.vector.reciprocal(rec[:], denoms[:])
                nc.vector.tensor_copy(recb[0:16, :], rec[:])
                norm1_and_outproj(0)
                attn_group(1, 2, stq, otq)
                norm_pair0(2)
                norm1_and_outproj(1)
                attn_group(1, 3, stq, otq)
                norm_pair0(3)
                nc.sync.dma_start(denoms[12:16, :], dsum[0:1, 12:16, :])
                nc.vector.reciprocal(rec[:], denoms[:])
                nc.vector.tensor_copy(recb[0:16, :], rec[:])
                norm1_and_outproj(2)
                norm1_and_outproj(3)

    nc.compile()
    return nc


def _rot2(cols):
    """rotate_every_two on the column axis of a [*, 64] block: returns the 32
    rotated columns [-c1, c0, -c3, c2, ...]."""
    out = np.zeros_like(cols[:, :ROT])
    out[:, 0::2] = -cols[:, 1:ROT:2]
    out[:, 1::2] = cols[:, 0:ROT:2]
    return out


def _chunked(w):
    """[1024, 128] -> [128, 1024] with chunk-c cols at c*128."""
    return np.ascontiguousarray(w.reshape(8, 128, 128).transpose(1, 0, 2).reshape(128, 1024))


def _prep_host(s_q, s_kv, mask_q, mask_kv, Wq, bq_, Wkv, bkv_, Wo, bo_):
    inv_freq = 1.0 / (10000.0 ** (np.arange(0, ROT, 2, dtype=np.float64) / ROT))
    t = np.arange(N, dtype=np.float64)[None, :] * inv_freq[:, None]  # [16, N]
    cos32 = np.repeat(np.cos(t), 2, axis=0).astype(NPBF16)  # [32, N]
    sin32 = np.repeat(np.sin(t), 2, axis=0).astype(NPBF16)
    cosT = np.zeros((128, N), NPBF16)
    sinT = np.zeros((128, N), NPBF16)
    for r in (0, 64):
        cosT[r : r + 32] = cos32
    for r in (0, 32, 64, 96):
        sinT[r : r + 32] = sin32

    # lower-triangular keep-mask (key row <= query col), applied to exp(p)
    pidx = np.arange(128)
    mt = (pidx[:, None] <= pidx[None, :]).astype(np.float32)
    mtile2 = np.stack([mt, mt], axis=1).astype(NPBF16)  # [128, 2, 128]

    e64 = np.zeros((64, 8, 128), NPBF16)
    for pr in range(2):
        for g in range(NG):
            e64[pr * 8 + g * 2 + 0, pr * 4 + g, 0:64] = 1.0
            e64[pr * 8 + g * 2 + 1, pr * 4 + g, 64:128] = 1.0
    e64 = e64.reshape(64, 1024)

    in_maps = []
    for core in range(8):
        b = core // 4
        h0 = (core % 4) * HPC

        wq = np.zeros((2, 128, 1024), NPBF16)
        wk = np.zeros((2, 128, 1024), NPBF16)
        bqp = np.zeros((128, 2), np.float32)
        bkp = np.zeros((128, 2), np.float32)
        wq2 = np.zeros((1024, 128), np.float32)
        wk2 = np.zeros((1024, 128), np.float32)
        bq2 = np.zeros((128, 1), np.float32)
        bk2 = np.zeros((128, 1), np.float32)
        for pr in range(2):
            cols_q, cols_k, bq_c, bk_c = [], [], [], []
            for hh in range(2):
                h = h0 + 2 * pr + hh
                qcols = Wq[:, h * DH : (h + 1) * DH]
                kcols = Wkv[:, h * 2 * DH : h * 2 * DH + DH]
                cols_q.append(qcols)
                bq_c.append(bq_[h * DH : (h + 1) * DH])
                cols_k.append(kcols)
                bk_c.append(bkv_[h * 2 * DH : h * 2 * DH + DH])
                r2 = 64 * pr + 32 * hh
                wq2[:, r2 : r2 + 32] = _rot2(qcols)
                wk2[:, r2 : r2 + 32] = _rot2(kcols)
                bq2[r2 : r2 + 32, 0] = _rot2(bq_[h * DH : (h + 1) * DH][None, :])[0]
                bk2[r2 : r2 + 32, 0] = _rot2(
                    bkv_[h * 2 * DH : h * 2 * DH + DH][None, :]
                )[0]
            wq[pr] = _chunked(np.concatenate(cols_q, axis=1)).astype(NPBF16)
            wk[pr] = _chunked(np.concatenate(cols_k, axis=1)).astype(NPBF16)
            bqp[:, pr] = np.concatenate(bq_c)
            bkp[:, pr] = np.concatenate(bk_c)

        # wv: [128, chunk(8) x (pr,hh,dim)(256)]
        wv = np.zeros((8, 128, 256), np.float32)
        for pr in range(2):
            for hh in range(2):
                h = h0 + 2 * pr + hh
                vcols = Wkv[:, h * 2 * DH + DH : (h + 1) * 2 * DH]  # [1024, 64]
                wv[:, :, (pr * 2 + hh) * 64 : (pr * 2 + hh + 1) * 64] = vcols.reshape(
                    8, 128, 64
                )
        wv = np.ascontiguousarray(wv.transpose(1, 0, 2).reshape(128, 2048)).astype(NPBF16)

        wo_rows = Wo[h0 * DH : (h0 + HPC) * DH, :]  # [256, 1024]
        bmask = np.tile(
            (INF * (mask_kv[b].astype(np.float32) - 1.0)).reshape(NB, 128).T[:, :],
            (1, 1),
        )  # [128, NB]

        in_maps.append(
            {
                "sqt": np.ascontiguousarray(s_q[b].T).astype(NPBF16),
                "skvt": np.ascontiguousarray(s_kv[b].T).astype(NPBF16),
                "wq": wq,
                "wk": wk,
                "wq2": _chunked(wq2).astype(NPBF16),
                "wk2": _chunked(wk2).astype(NPBF16),
                "bq2": bq2,
                "bk2": bk2,
                "wv": wv,
                "wo": np.ascontiguousarray(wo_rows.reshape(2, 128, DOUT)).astype(NPBF16),
                "bq": bqp,
                "bk": bkp,
                "cost": cosT,
                "sint": sinT,
                "mtile": mtile2,
                "e64": e64,
                "bmask": np.ascontiguousarray(bmask).astype(np.float32),
            }
        )
    return in_maps


_NC_CACHE = {}


def kernel(s_q, s_kv, mask_q, mask_kv, Wq, bq, Wkv, bkv, Wo, bo, _return_results=False):
    from concourse.bass_utils import run_bass_kernel_spmd

    if "nc" not in _NC_CACHE:
        _NC_CACHE["nc"] = build_nc()
    nc = _NC_CACHE["nc"]

    s_q = np.asarray(s_q, np.float32)
    s_kv = np.asarray(s_kv, np.float32)
    Wq_ = np.asarray(Wq, np.float32)
    Wkv_ = np.asarray(Wkv, np.float32)
    Wo_ = np.asarray(Wo, np.float32)
    bkv_ = np.asarray(bkv, np.float32)
    in_maps = _prep_host(
        s_q, s_kv,
        np.asarray(mask_q, np.float32),
        np.asarray(mask_kv, np.float32),
        Wq_, np.asarray(bq, np.float32), Wkv_, bkv_, Wo_, np.asarray(bo, np.float32),
    )
    trace = bool(int(os.environ.get("KERNEL_TRACE", "0")))
    res = run_bass_kernel_spmd(nc, in_maps, core_ids=list(range(8)), trace=trace)

    # v-bias contribution folded here: softmax weights sum to 1, so each
    # head's o_norm is missing exactly +bv; add bv_full @ Wo once per batch.
    bv_full = bkv_.reshape(H, 2 * DH)[:, DH:].reshape(-1)
    bo_eff = np.asarray(bo, np.float32) + bv_full @ Wo_

    out = np.zeros((B, N, DOUT), np.float32)
    for core in range(8):
        b = core // 4
        out[b] += res.results[core]["out"].astype(np.float32)
    out += bo_eff[None, None, :]
    if _return_results:
        return out, res
    return out
